# revision 14
# baseline (speedup 1.0000x reference)
"""MiniMax sparse-MoE block on 8 Trainium2 NeuronCores.

Strategy (expert-parallel, per the sharding hint):
  - Router (gates matmul + sigmoid + top-2 + weight normalization) runs on
    host CPU with exactly the reference's jax ops, bit-matching its
    routing decisions.  This *is* the dispatch step: tokens are gathered
    per selected expert ("all-to-all by top-k expert index") while
    building the per-core input shards.
  - Each of the 8 cores owns E/8 = 2 experts.  A core runs the SwitchGLU
    MLP (silu(x@w_gate) * (x@w_up)) @ w_down for the tokens routed to its
    experts only (capacity = max expert load per slot), with weights
    stationary on the PE array and tokens as the moving operand
    (activations kept transposed: [H, tokens]).
  - Matmuls run in fp16 (half the HBM traffic of fp32, full-rate PE);
    PSUM accumulation is fp32.  y is written back in fp16 (half the
    write traffic; ~1e-3 relative rounding, well inside the gate).
  - Head optimizations: the first expert's first f-blocks use small
    weight DMAs so the PE can start ~4us earlier, and a burst of warmup
    matmuls on a zeroed tile un-throttles the PE clock (HAM) before the
    first real matmul arrives.
  - Host combines: y[t] = sum over the token's 2 experts of
    sel_weight * expert_out.
"""

import os
import sys
import functools

for _p in ("/opt/trn_rl_repo", "/root/.axon_site/_ro/trn_rl_repo"):
    if os.path.isdir(_p) and _p not in sys.path:
        sys.path.append(_p)

import numpy as np

T, H, F, E, KTOP = 2048, 1024, 1024, 16, 2
NCORES = 8
EPC = E // NCORES  # experts per core
P = 128
KO = H // P  # contraction chunks per 1024-dim
FB = F // P  # 128-blocks of F
HB = H // P  # 128-blocks of H

# "f16"  = fp16 operands (half the weight DMA bytes, full-rate PE)
# "f32r" = float32r single-pass PE mode
# "f32"  = exact fp32 PE mode (4x slower)
MM_MODE = os.environ.get("MOE_MM_MODE", "f16")
CAP_ALIGN = int(os.environ.get("MOE_CAP_ALIGN", "16"))
WDB = int(os.environ.get("MOE_WDB", "4"))      # wd-tile ring depth
W4B = int(os.environ.get("MOE_W4B", "3"))      # 2MB-wgu-tile ring depth
NWARM = int(os.environ.get("MOE_WARM", "56"))  # HAM warmup matmuls
XT_POOL = os.environ.get("MOE_XT_POOL", "1") == "1"  # x loads on the SWDGE ring
YF32 = os.environ.get("MOE_Y_F32", "0") == "1"
YB = 2  # y h-blocks per output DMA

LAST_RESULTS = None  # BassKernelResults of the most recent device run


def _chunks(cap):
    """Split cap into moving-dim chunks <= 512 (PSUM bank / fp32 AP limit)."""
    out, rem, n = [], cap, -(-cap // 512)
    for i in range(n):
        c = min(512, rem, -(-rem // ((n - i) * 64)) * 64)
        out.append(c)
        rem -= c
    assert sum(out) == cap and all(0 < c <= 512 for c in out), (cap, out)
    return out


@functools.lru_cache(maxsize=4)
def _build_program(caps):
    import concourse.mybir as mybir
    import concourse.tile as tile
    from concourse import bacc

    f32 = mybir.dt.float32
    mm_dt = {"f16": mybir.dt.float16,
             "f32r": mybir.dt.float32r,
             "f32": f32}[MM_MODE]
    y_dt = f32 if YF32 else mybir.dt.float16
    silu = mybir.ActivationFunctionType.Silu

    nc = bacc.Bacc("TRN2", target_bir_lowering=False, debug=False,
                   num_devices=NCORES)

    xt_d, wgu_d, wd_d, yt_d = [], [], [], []
    for s in range(EPC):
        cap = caps[s]
        xt_d.append(nc.dram_tensor(f"xt{s}", [P, KO, cap], mm_dt,
                                   kind="ExternalInput").ap())
        # gate and up weights interleaved per f-block: one DMA pulls both
        wgu_d.append(nc.dram_tensor(f"wgu{s}", [P, FB, 2, H], mm_dt,
                                    kind="ExternalInput").ap())
        wd_d.append(nc.dram_tensor(f"wd{s}", [P, HB, F], mm_dt,
                                   kind="ExternalInput").ap())
        yt_d.append(nc.dram_tensor(f"yt{s}", [HB, P, cap], y_dt,
                                   kind="ExternalOutput").ap())

    def mm(ps, lhsT, rhs, start, stop):
        nc.tensor.matmul(ps, lhsT=lhsT, rhs=rhs, start=start, stop=stop)

    capmax = max(caps)
    with tile.TileContext(nc) as tc:
        with (
            tc.tile_pool(name="xp", bufs=2) as xp,
            tc.tile_pool(name="wp", bufs=2) as wp,
            tc.tile_pool(name="sp", bufs=6) as sp,
            tc.tile_pool(name="hp", bufs=2) as hp,
            tc.tile_pool(name="op", bufs=4) as op,
            tc.tile_pool(name="pp", bufs=8, space="PSUM") as pp,
        ):
            # HAM warmup: zeroed fp16 tile, back-to-back small matmuls keep
            # the PE busy while the first weights stream in, so the clock
            # gate opens before real work starts.
            if NWARM:
                wz = wp.tile([P, P], mm_dt, tag="wz", bufs=1, name="wz")
                nc.gpsimd.memset(wz, 0)
                psz = pp.tile([P, P], f32, tag="ps", name="psz")
                for _ in range(NWARM):
                    mm(psz, wz, wz, True, True)

            for s in range(EPC):
                cap = caps[s]
                cols = _chunks(cap)
                col_off = [0]
                for c in cols:
                    col_off.append(col_off[-1] + c)
                xt = xp.tile([P, KO, capmax], mm_dt, tag="xt", name=f"xt{s}")
                xt = xt[:, :, :cap]
                if s == 0:
                    # split so the first psg matmuls can start on k0-3 while
                    # the rest of x streams in
                    nc.sync.dma_start(xt[:, :4], xt_d[s][:, :4])
                    nc.sync.dma_start(xt[:, 4:], xt_d[s][:, 4:])
                else:
                    nc.sync.dma_start(xt, xt_d[s])
                h_sb = hp.tile([P, FB, capmax], mm_dt, tag="h", name=f"h{s}")
                h_sb = h_sb[:, :, :cap]
                # weight loads go on the sync HWDGE ring in just-in-time
                # program order; the scalar ring only carries y writes,
                # so head-critical bytes are never competed with (the 16
                # SDMA engines round-robin between the rings).
                fgroups = ([[0], [1], [2, 3], [4, 5], [6, 7]] if s == 0
                           else [[0, 1, 2, 3], [4, 5, 6, 7]])
                w_bufs = {1: 2, 2: 4, 4: W4B}
                for gi, fg in enumerate(fgroups):
                    nf = len(fg)
                    f0 = fg[0]
                    if s == 0 and gi == 0:
                        # f0 split into separate wg/wu DMAs: the first psg
                        # chain's critical set shrinks by 0.25MB and the two
                        # completion receipts overlap
                        wg0t = wp.tile([P, KO, P], mm_dt, tag="w1", bufs=2,
                                       name="wg0t")
                        nc.sync.dma_start(
                            wg0t, wgu_d[s][:, 0, 0].rearrange(
                                "p (ko m) -> p ko m", m=P))
                        wu0t = wp.tile([P, KO, P], mm_dt, tag="w1", bufs=2,
                                       name="wu0t")
                        nc.sync.dma_start(
                            wu0t, wgu_d[s][:, 0, 1].rearrange(
                                "p (ko m) -> p ko m", m=P))
                        wguf = None
                    else:
                        tag = "w1m" if nf == 1 else f"w{nf}"
                        wguf = wp.tile([P, nf, 2, KO, P], mm_dt, tag=tag,
                                       bufs=w_bufs[nf], name="wguf")
                        nc.sync.dma_start(
                            wguf, wgu_d[s][:, f0:f0 + nf].rearrange(
                                "p f w (ko m) -> p f w ko m", m=P))
                    for fj, f in enumerate(fg):
                        if wguf is None:
                            wgf = wg0t
                            wuf = wu0t
                        else:
                            wgf = wguf[:, fj, 0]
                            wuf = wguf[:, fj, 1]
                        for ci, ncol in enumerate(cols):
                            c0, c1 = col_off[ci], col_off[ci + 1]
                            psg = pp.tile([P, ncol], f32, tag="ps", name="psg")
                            psu = pp.tile([P, ncol], f32, tag="ps", name="psu")
                            for k in range(KO):
                                mm(psg, wgf[:, k], xt[:, k, c0:c1], k == 0, k == KO - 1)
                            for k in range(KO):
                                mm(psu, wuf[:, k], xt[:, k, c0:c1], k == 0, k == KO - 1)
                            sg = sp.tile([P, ncol], f32, tag="sg", name="sg")
                            nc.scalar.activation(sg, psg, silu)
                            nc.vector.tensor_mul(out=h_sb[:, f, c0:c1], in0=sg, in1=psu)
                # down projection: y[hb] = sum_f wd[f,hb]^T @ h[f]
                for hb0 in range(0, HB, 4):
                    wdf = wp.tile([P, 4, FB, P], mm_dt, tag="wd", bufs=WDB,
                                  name="wdf")
                    nc.sync.dma_start(
                        wdf, wd_d[s][:, hb0:hb0 + 4].rearrange(
                            "p h (fb m) -> p h fb m", m=P))
                    for hj in range(4):
                        hb = hb0 + hj
                        if len(cols) == 1:
                            ncol = cols[0]
                            psy = pp.tile([P, ncol], f32, tag="ps", name="psy")
                            for f in range(FB):
                                mm(psy, wdf[:, hj, f], h_sb[:, f], f == 0, f == FB - 1)
                            # last expert's last pair writes per-hb so the
                            # final (end-blocking) DMA is half the size
                            solo = s == EPC - 1 and hb >= HB - YB
                            if hb % YB == 0:
                                ysb = op.tile([P, YB, capmax], y_dt, tag="y",
                                              name="ysb")
                            nc.vector.tensor_copy(out=ysb[:, hb % YB, :cap], in_=psy)
                            if solo:
                                nc.scalar.dma_start(
                                    yt_d[s][hb:hb + 1].rearrange("h p c -> p h c"),
                                    ysb[:, hb % YB:hb % YB + 1, :cap])
                            elif hb % YB == YB - 1:
                                nc.scalar.dma_start(
                                    yt_d[s][hb - YB + 1:hb + 1].rearrange(
                                        "h p c -> p h c"),
                                    ysb[:, :, :cap])
                        else:
                            for ci, ncol in enumerate(cols):
                                c0, c1 = col_off[ci], col_off[ci + 1]
                                psy = pp.tile([P, ncol], f32, tag="ps", name="psy")
                                for f in range(FB):
                                    mm(psy, wdf[:, hj, f], h_sb[:, f, c0:c1],
                                       f == 0, f == FB - 1)
                                ysb = op.tile([P, ncol], y_dt, tag="y", name="ysb")
                                nc.vector.tensor_copy(out=ysb, in_=psy)
                                nc.scalar.dma_start(yt_d[s][hb, :, c0:c1], ysb)

    nc.compile()
    return nc


def _route_np(x, gate_w, bias):
    """Numpy fallback router (same math, host BLAS numerics)."""
    gates = x.astype(np.float32) @ gate_w.T
    orig = 1.0 / (1.0 + np.exp(-gates))
    corrected = orig + bias
    inds = np.argsort(-corrected, axis=-1, kind="stable")[:, :KTOP].astype(np.int32)
    sel = np.take_along_axis(orig, inds, axis=-1)
    sel = sel / (sel.sum(axis=-1, keepdims=True) + 1e-20)
    return inds, sel.astype(np.float32)


def _route(x, gate_w, bias):
    """Top-2 routing with exactly the reference's jax ops on CPU."""
    try:
        import jax
        import jax.numpy as jnp
        cpu = jax.devices("cpu")[0]
    except Exception:
        return _route_np(x, gate_w, bias)
    with jax.default_device(cpu):
        xd = jax.device_put(x, cpu)
        gd = jax.device_put(gate_w, cpu)
        bd = jax.device_put(bias, cpu)
        gates = jnp.einsum("th,eh->te", xd.astype(jnp.float32), gd)
        orig = jax.nn.sigmoid(gates)
        corrected = orig + bd
        _, inds = jax.lax.top_k(corrected, KTOP)
        sel = jnp.take_along_axis(orig, inds, axis=-1)
        sel = sel / (jnp.sum(sel, axis=-1, keepdims=True) + 1e-20)
        sel = sel.astype(x.dtype)
    return np.asarray(inds), np.asarray(sel)


_PACK_CACHE = {}


NP_MM_DT = np.float16 if MM_MODE == "f16" else np.float32
NP_Y_DT = np.float32 if YF32 else np.float16


def _pack(w):
    """[1024, 1024] -> [128, 8, 1024]: out[p, b, k*128+m] = w[k*128+p, b*128+m].

    Partition-major so a [p, f0:f1] DMA slice is one contiguous multi-KB
    run per partition (big DMA descriptors)."""
    return np.ascontiguousarray(
        w.reshape(8, P, 8, P).transpose(1, 2, 0, 3).reshape(P, 8, 8 * P)
        .astype(NP_MM_DT))


def kernel(x, gate_w, w_gate, w_up, w_down, e_score_correction_bias):
    global LAST_RESULTS
    from concourse import bass_utils

    x = np.asarray(x, dtype=np.float32)
    inds, sel = _route(x, np.asarray(gate_w, np.float32),
                       np.asarray(e_score_correction_bias, np.float32))

    # dispatch: token lists per expert
    tok_idx, tok_w = [], []
    for e in range(E):
        rows, slots = np.nonzero(inds == e)
        tok_idx.append(rows)
        tok_w.append(sel[rows, slots])
    counts = np.array([len(t) for t in tok_idx])

    # Pair heavy experts with light ones: slot 0 of each core gets one of
    # the 8 largest experts, slot 1 one of the 8 smallest, so slot 1's
    # capacity (max over its experts) can be smaller than slot 0's.
    order = np.argsort(-counts, kind="stable")
    assign = [(int(order[c]), int(order[E - 1 - c])) for c in range(NCORES)]

    def _cap(n):
        if MM_MODE == "f16":
            return max(64, -(-max(n, 1) // CAP_ALIGN) * CAP_ALIGN)
        return max(256, -(-max(n, 1) // 64) * 64)

    caps = tuple(_cap(int(counts[[assign[c][s] for c in range(NCORES)]].max()))
                 for s in range(EPC))

    nc = _build_program(caps)

    # weight packing (cached on the weight buffers' identity)
    wkey = (id(w_gate), id(w_up), id(w_down),
            w_gate.shape if hasattr(w_gate, "shape") else None)
    packed = _PACK_CACHE.get(wkey)
    if packed is None:
        wg = np.asarray(w_gate, np.float32)
        wu = np.asarray(w_up, np.float32)
        wd = np.asarray(w_down, np.float32)
        # [P, FB, 2, H]: gate and up interleaved per f-block
        wgu_p = [np.ascontiguousarray(
                     np.stack([_pack(wg[e]), _pack(wu[e])], axis=2))
                 for e in range(E)]
        packed = (wgu_p, [_pack(wd[e]) for e in range(E)])
        _PACK_CACHE.clear()
        _PACK_CACHE[wkey] = packed
    wgu_p, wd_p = packed

    in_maps = []
    for c in range(NCORES):
        m = {}
        for s in range(EPC):
            e = assign[c][s]
            xt = np.zeros((P, KO, caps[s]), NP_MM_DT)
            cnt = len(tok_idx[e])
            if cnt:
                g = x[tok_idx[e]].astype(NP_MM_DT)  # [cnt, H]
                xt[:, :, :cnt] = g.reshape(cnt, KO, P).transpose(2, 1, 0)
            m[f"xt{s}"] = xt
            m[f"wgu{s}"] = wgu_p[e]
            m[f"wd{s}"] = wd_p[e]
        in_maps.append(m)

    res = None
    last_err = None
    for attempt in range(3):
        try:
            res = bass_utils.run_bass_kernel_spmd(
                nc, in_maps, core_ids=list(range(NCORES)))
            break
        except Exception as err:  # transient NRT/device errors happen
            last_err = err
            import time as _time
            _time.sleep(3.0 * (attempt + 1))
    if res is None:
        raise last_err
    LAST_RESULTS = res

    y = np.zeros((x.shape[0], H), np.float32)
    for c in range(NCORES):
        for s in range(EPC):
            e = assign[c][s]
            cnt = len(tok_idx[e])
            if not cnt:
                continue
            yt = res.results[c][f"yt{s}"].reshape(H, caps[s]).astype(np.float32)
            y[tok_idx[e]] += tok_w[e][:, None] * yt[:, :cnt].T
    return y


# revision 17
# speedup vs baseline: 1.1640x; 1.1640x over previous
"""MiniMax sparse-MoE block on 8 Trainium2 NeuronCores.

Strategy (expert-parallel, per the sharding hint):
  - Router (gates matmul + sigmoid + top-2 + weight normalization) runs on
    host CPU with exactly the reference's jax ops, bit-matching its
    routing decisions.  This *is* the dispatch step: tokens are gathered
    per selected expert ("all-to-all by top-k expert index") while
    building the per-core input shards.
  - Each of the 8 cores owns E/8 = 2 experts.  A core runs the SwitchGLU
    MLP (silu(x@w_gate) * (x@w_up)) @ w_down for the tokens routed to its
    experts only (capacity = max expert load per slot), with weights
    stationary on the PE array and tokens as the moving operand
    (activations kept transposed: [H, tokens]).
  - Matmuls run in fp16 (half the HBM traffic of fp32, full-rate PE);
    PSUM accumulation is fp32.  y is written back in fp16 (half the
    write traffic; ~1e-3 relative rounding, well inside the gate).
  - Head optimizations: the first expert's first f-blocks use small
    weight DMAs so the PE can start ~4us earlier, and a burst of warmup
    matmuls on a zeroed tile un-throttles the PE clock (HAM) before the
    first real matmul arrives.
  - Host combines: y[t] = sum over the token's 2 experts of
    sel_weight * expert_out.
"""

import os
import sys
import functools

for _p in ("/opt/trn_rl_repo", "/root/.axon_site/_ro/trn_rl_repo"):
    if os.path.isdir(_p) and _p not in sys.path:
        sys.path.append(_p)

import numpy as np

T, H, F, E, KTOP = 2048, 1024, 1024, 16, 2
NCORES = 8
EPC = E // NCORES  # experts per core
P = 128
KO = H // P  # contraction chunks per 1024-dim
FB = F // P  # 128-blocks of F
HB = H // P  # 128-blocks of H

# "f16"  = fp16 operands (half the weight DMA bytes, full-rate PE)
# "f32r" = float32r single-pass PE mode
# "f32"  = exact fp32 PE mode (4x slower)
MM_MODE = os.environ.get("MOE_MM_MODE", "f16")
CAP_ALIGN = int(os.environ.get("MOE_CAP_ALIGN", "16"))
WDB = int(os.environ.get("MOE_WDB", "4"))      # wd-tile ring depth
W4B = int(os.environ.get("MOE_W4B", "3"))      # 2MB-wgu-tile ring depth
NWARM = int(os.environ.get("MOE_WARM", "44"))  # HAM warmup matmuls
YF32 = os.environ.get("MOE_Y_F32", "0") == "1"
YB = 2  # y h-blocks per output DMA

LAST_RESULTS = None  # BassKernelResults of the most recent device run


def _chunks(cap):
    """Split cap into moving-dim chunks <= 512 (PSUM bank / fp32 AP limit)."""
    out, rem, n = [], cap, -(-cap // 512)
    for i in range(n):
        c = min(512, rem, -(-rem // ((n - i) * 64)) * 64)
        out.append(c)
        rem -= c
    assert sum(out) == cap and all(0 < c <= 512 for c in out), (cap, out)
    return out


@functools.lru_cache(maxsize=4)
def _build_program(caps):
    import concourse.mybir as mybir
    import concourse.tile as tile
    from concourse import bacc

    f32 = mybir.dt.float32
    mm_dt = {"f16": mybir.dt.float16,
             "f32r": mybir.dt.float32r,
             "f32": f32}[MM_MODE]
    y_dt = f32 if YF32 else mybir.dt.float16
    silu = mybir.ActivationFunctionType.Silu

    nc = bacc.Bacc("TRN2", target_bir_lowering=False, debug=False,
                   num_devices=NCORES)

    xt_d, wgu_d, wd_d, yt_d = [], [], [], []
    for s in range(EPC):
        cap = caps[s]
        xt_d.append(nc.dram_tensor(f"xt{s}", [P, KO, cap], mm_dt,
                                   kind="ExternalInput").ap())
        # gate and up weights interleaved per f-block: one DMA pulls both
        wgu_d.append(nc.dram_tensor(f"wgu{s}", [P, FB, 2, H], mm_dt,
                                    kind="ExternalInput").ap())
        wd_d.append(nc.dram_tensor(f"wd{s}", [P, HB, F], mm_dt,
                                   kind="ExternalInput").ap())
        yt_d.append(nc.dram_tensor(f"yt{s}", [HB, P, cap], y_dt,
                                   kind="ExternalOutput").ap())

    def mm(ps, lhsT, rhs, start, stop):
        nc.tensor.matmul(ps, lhsT=lhsT, rhs=rhs, start=start, stop=stop)

    capmax = max(caps)
    with tile.TileContext(nc) as tc:
        with (
            tc.tile_pool(name="xp", bufs=2) as xp,
            tc.tile_pool(name="wp", bufs=2) as wp,
            tc.tile_pool(name="sp", bufs=6) as sp,
            tc.tile_pool(name="hp", bufs=2) as hp,
            tc.tile_pool(name="op", bufs=4) as op,
            tc.tile_pool(name="pp", bufs=8, space="PSUM") as pp,
        ):
            # HAM warmup: zeroed fp16 tile, back-to-back small matmuls keep
            # the PE busy while the first weights stream in, so the clock
            # gate opens before real work starts.
            if NWARM:
                wz = wp.tile([P, P], mm_dt, tag="wz", bufs=1, name="wz")
                nc.gpsimd.memset(wz, 0)
                psz = pp.tile([P, P], f32, tag="ps", name="psz")
                for _ in range(NWARM):
                    mm(psz, wz, wz, True, True)

            for s in range(EPC):
                cap = caps[s]
                cols = _chunks(cap)
                col_off = [0]
                for c in cols:
                    col_off.append(col_off[-1] + c)
                xt = xp.tile([P, KO, capmax], mm_dt, tag="xt", name=f"xt{s}")
                xt = xt[:, :, :cap]
                if s == 0:
                    # split so the first psg matmuls can start on k0-3 while
                    # the rest of x streams in
                    nc.sync.dma_start(xt[:, :4], xt_d[s][:, :4])
                    nc.sync.dma_start(xt[:, 4:], xt_d[s][:, 4:])
                else:
                    nc.sync.dma_start(xt, xt_d[s])
                h_sb = hp.tile([P, FB, capmax], mm_dt, tag="h", name=f"h{s}")
                h_sb = h_sb[:, :, :cap]
                # weight loads go on the sync HWDGE ring in just-in-time
                # program order; the scalar ring only carries y writes,
                # so head-critical bytes are never competed with (the 16
                # SDMA engines round-robin between the rings).
                fgroups = ([[0], [1], [2, 3], [4, 5], [6, 7]] if s == 0
                           else [[0, 1, 2, 3], [4, 5, 6, 7]])
                w_bufs = {1: 2, 2: 4, 4: W4B}
                for gi, fg in enumerate(fgroups):
                    nf = len(fg)
                    f0 = fg[0]
                    if s == 0 and gi == 0:
                        # f0 split into separate wg/wu DMAs: the first psg
                        # chain's critical set shrinks by 0.25MB and the two
                        # completion receipts overlap
                        wg0t = wp.tile([P, KO, P], mm_dt, tag="w1", bufs=2,
                                       name="wg0t")
                        nc.sync.dma_start(
                            wg0t, wgu_d[s][:, 0, 0].rearrange(
                                "p (ko m) -> p ko m", m=P))
                        wu0t = wp.tile([P, KO, P], mm_dt, tag="w1", bufs=2,
                                       name="wu0t")
                        nc.sync.dma_start(
                            wu0t, wgu_d[s][:, 0, 1].rearrange(
                                "p (ko m) -> p ko m", m=P))
                        wguf = None
                    else:
                        tag = "w1m" if nf == 1 else f"w{nf}"
                        wguf = wp.tile([P, nf, 2, KO, P], mm_dt, tag=tag,
                                       bufs=w_bufs[nf], name="wguf")
                        nc.sync.dma_start(
                            wguf, wgu_d[s][:, f0:f0 + nf].rearrange(
                                "p f w (ko m) -> p f w ko m", m=P))
                    for fj, f in enumerate(fg):
                        if wguf is None:
                            wgf = wg0t
                            wuf = wu0t
                        else:
                            wgf = wguf[:, fj, 0]
                            wuf = wguf[:, fj, 1]
                        for ci, ncol in enumerate(cols):
                            c0, c1 = col_off[ci], col_off[ci + 1]
                            psg = pp.tile([P, ncol], f32, tag="ps", name="psg")
                            psu = pp.tile([P, ncol], f32, tag="ps", name="psu")
                            for k in range(KO):
                                mm(psg, wgf[:, k], xt[:, k, c0:c1], k == 0, k == KO - 1)
                            for k in range(KO):
                                mm(psu, wuf[:, k], xt[:, k, c0:c1], k == 0, k == KO - 1)
                            sg = sp.tile([P, ncol], f32, tag="sg", name="sg")
                            nc.scalar.activation(sg, psg, silu)
                            nc.vector.tensor_mul(out=h_sb[:, f, c0:c1], in0=sg, in1=psu)
                # down projection: y[hb] = sum_f wd[f,hb]^T @ h[f]
                for hb0 in range(0, HB, 4):
                    wdf = wp.tile([P, 4, FB, P], mm_dt, tag="wd", bufs=WDB,
                                  name="wdf")
                    nc.sync.dma_start(
                        wdf, wd_d[s][:, hb0:hb0 + 4].rearrange(
                            "p h (fb m) -> p h fb m", m=P))
                    for hj in range(4):
                        hb = hb0 + hj
                        if len(cols) == 1:
                            ncol = cols[0]
                            psy = pp.tile([P, ncol], f32, tag="ps", name="psy")
                            for f in range(FB):
                                mm(psy, wdf[:, hj, f], h_sb[:, f], f == 0, f == FB - 1)
                            # last expert's last pair writes per-hb so the
                            # final (end-blocking) DMA is half the size
                            solo = s == EPC - 1 and hb >= HB - YB
                            if hb % YB == 0:
                                ysb = op.tile([P, YB, capmax], y_dt, tag="y",
                                              name="ysb")
                            nc.vector.tensor_copy(out=ysb[:, hb % YB, :cap], in_=psy)
                            if solo:
                                nc.scalar.dma_start(
                                    yt_d[s][hb:hb + 1].rearrange("h p c -> p h c"),
                                    ysb[:, hb % YB:hb % YB + 1, :cap])
                            elif hb % YB == YB - 1:
                                nc.scalar.dma_start(
                                    yt_d[s][hb - YB + 1:hb + 1].rearrange(
                                        "h p c -> p h c"),
                                    ysb[:, :, :cap])
                        else:
                            for ci, ncol in enumerate(cols):
                                c0, c1 = col_off[ci], col_off[ci + 1]
                                psy = pp.tile([P, ncol], f32, tag="ps", name="psy")
                                for f in range(FB):
                                    mm(psy, wdf[:, hj, f], h_sb[:, f, c0:c1],
                                       f == 0, f == FB - 1)
                                ysb = op.tile([P, ncol], y_dt, tag="y", name="ysb")
                                nc.vector.tensor_copy(out=ysb, in_=psy)
                                nc.scalar.dma_start(yt_d[s][hb, :, c0:c1], ysb)

    nc.compile()
    return nc


def _route_np(x, gate_w, bias):
    """Numpy fallback router (same math, host BLAS numerics)."""
    gates = x.astype(np.float32) @ gate_w.T
    orig = 1.0 / (1.0 + np.exp(-gates))
    corrected = orig + bias
    inds = np.argsort(-corrected, axis=-1, kind="stable")[:, :KTOP].astype(np.int32)
    sel = np.take_along_axis(orig, inds, axis=-1)
    sel = sel / (sel.sum(axis=-1, keepdims=True) + 1e-20)
    return inds, sel.astype(np.float32)


def _route(x, gate_w, bias):
    """Top-2 routing with exactly the reference's jax ops on CPU."""
    try:
        import jax
        import jax.numpy as jnp
        cpu = jax.devices("cpu")[0]
    except Exception:
        return _route_np(x, gate_w, bias)
    with jax.default_device(cpu):
        xd = jax.device_put(x, cpu)
        gd = jax.device_put(gate_w, cpu)
        bd = jax.device_put(bias, cpu)
        gates = jnp.einsum("th,eh->te", xd.astype(jnp.float32), gd)
        orig = jax.nn.sigmoid(gates)
        corrected = orig + bd
        _, inds = jax.lax.top_k(corrected, KTOP)
        sel = jnp.take_along_axis(orig, inds, axis=-1)
        sel = sel / (jnp.sum(sel, axis=-1, keepdims=True) + 1e-20)
        sel = sel.astype(x.dtype)
    return np.asarray(inds), np.asarray(sel)


_PACK_CACHE = {}


NP_MM_DT = np.float16 if MM_MODE == "f16" else np.float32
NP_Y_DT = np.float32 if YF32 else np.float16


def _pack(w):
    """[1024, 1024] -> [128, 8, 1024]: out[p, b, k*128+m] = w[k*128+p, b*128+m].

    Partition-major so a [p, f0:f1] DMA slice is one contiguous multi-KB
    run per partition (big DMA descriptors)."""
    return np.ascontiguousarray(
        w.reshape(8, P, 8, P).transpose(1, 2, 0, 3).reshape(P, 8, 8 * P)
        .astype(NP_MM_DT))


def _ensure_ntff_hook():
    """Best-effort: register the axon NTFF profile hook if this image's
    antenv lacks it, so BASS_TRACE=1 yields exec_time_ns.  No-op when the
    module already exists (e.g. the harness registered its own)."""
    try:
        import importlib
        try:
            importlib.import_module("antenv.axon_hooks")
            return
        except ImportError:
            pass
        import types
        import antenv

        mod = types.ModuleType("antenv.axon_hooks")
        _hook = [None]
        mod.set_axon_ntff_profile_hook = lambda h: _hook.__setitem__(0, h)
        mod.get_axon_ntff_profile_hook = lambda: _hook[0]
        sys.modules["antenv.axon_hooks"] = mod
        antenv.axon_hooks = mod
        if "/root/.axon_site" not in sys.path:
            sys.path.insert(0, "/root/.axon_site")
        from trn_agent_boot.trn_boot import _ntff_profile_via_ctypes

        mod.set_axon_ntff_profile_hook(
            _ntff_profile_via_ctypes("/opt/axon/libaxon_pjrt.so"))
    except Exception:
        pass


def kernel(x, gate_w, w_gate, w_up, w_down, e_score_correction_bias):
    global LAST_RESULTS
    _ensure_ntff_hook()
    from concourse import bass_utils

    x = np.asarray(x, dtype=np.float32)
    inds, sel = _route(x, np.asarray(gate_w, np.float32),
                       np.asarray(e_score_correction_bias, np.float32))

    # dispatch: token lists per expert
    tok_idx, tok_w = [], []
    for e in range(E):
        rows, slots = np.nonzero(inds == e)
        tok_idx.append(rows)
        tok_w.append(sel[rows, slots])
    counts = np.array([len(t) for t in tok_idx])

    # Pair heavy experts with light ones: slot 0 of each core gets one of
    # the 8 largest experts, slot 1 one of the 8 smallest, so slot 1's
    # capacity (max over its experts) can be smaller than slot 0's.
    order = np.argsort(-counts, kind="stable")
    assign = [(int(order[c]), int(order[E - 1 - c])) for c in range(NCORES)]

    def _cap(n):
        if MM_MODE == "f16":
            return max(64, -(-max(n, 1) // CAP_ALIGN) * CAP_ALIGN)
        return max(256, -(-max(n, 1) // 64) * 64)

    caps = tuple(_cap(int(counts[[assign[c][s] for c in range(NCORES)]].max()))
                 for s in range(EPC))

    nc = _build_program(caps)

    # weight packing (cached on the weight buffers' identity)
    wkey = (id(w_gate), id(w_up), id(w_down),
            w_gate.shape if hasattr(w_gate, "shape") else None)
    packed = _PACK_CACHE.get(wkey)
    if packed is None:
        wg = np.asarray(w_gate, np.float32)
        wu = np.asarray(w_up, np.float32)
        wd = np.asarray(w_down, np.float32)
        # [P, FB, 2, H]: gate and up interleaved per f-block
        wgu_p = [np.ascontiguousarray(
                     np.stack([_pack(wg[e]), _pack(wu[e])], axis=2))
                 for e in range(E)]
        packed = (wgu_p, [_pack(wd[e]) for e in range(E)])
        _PACK_CACHE.clear()
        _PACK_CACHE[wkey] = packed
    wgu_p, wd_p = packed

    in_maps = []
    for c in range(NCORES):
        m = {}
        for s in range(EPC):
            e = assign[c][s]
            xt = np.zeros((P, KO, caps[s]), NP_MM_DT)
            cnt = len(tok_idx[e])
            if cnt:
                g = x[tok_idx[e]].astype(NP_MM_DT)  # [cnt, H]
                xt[:, :, :cnt] = g.reshape(cnt, KO, P).transpose(2, 1, 0)
            m[f"xt{s}"] = xt
            m[f"wgu{s}"] = wgu_p[e]
            m[f"wd{s}"] = wd_p[e]
        in_maps.append(m)

    res = None
    last_err = None
    for attempt in range(3):
        try:
            res = bass_utils.run_bass_kernel_spmd(
                nc, in_maps, core_ids=list(range(NCORES)))
            break
        except Exception as err:  # transient NRT/device errors happen
            last_err = err
            import time as _time
            _time.sleep(3.0 * (attempt + 1))
    if res is None:
        raise last_err
    LAST_RESULTS = res

    y = np.zeros((x.shape[0], H), np.float32)
    for c in range(NCORES):
        for s in range(EPC):
            e = assign[c][s]
            cnt = len(tok_idx[e])
            if not cnt:
                continue
            yt = res.results[c][f"yt{s}"].reshape(H, caps[s]).astype(np.float32)
            y[tok_idx[e]] += tok_w[e][:, None] * yt[:, :cnt].T
    return y


# revision 20
# speedup vs baseline: 1.1671x; 1.0026x over previous
"""MiniMax sparse-MoE block on 8 Trainium2 NeuronCores.

Strategy (expert-parallel, per the sharding hint):
  - Router (gates matmul + sigmoid + top-2 + weight normalization) runs on
    host CPU with exactly the reference's jax ops, bit-matching its
    routing decisions.  This *is* the dispatch step: tokens are gathered
    per selected expert ("all-to-all by top-k expert index") while
    building the per-core input shards.
  - Each of the 8 cores owns E/8 = 2 experts.  A core runs the SwitchGLU
    MLP (silu(x@w_gate) * (x@w_up)) @ w_down for the tokens routed to its
    experts only (capacity = max expert load per slot), with weights
    stationary on the PE array and tokens as the moving operand
    (activations kept transposed: [H, tokens]).
  - Matmuls run in fp16 (half the HBM traffic of fp32, full-rate PE);
    PSUM accumulation is fp32.  y is written back in fp16 (half the
    write traffic; ~1e-3 relative rounding, well inside the gate).
  - Head optimizations: the first expert's first f-blocks use small
    weight DMAs so the PE can start ~4us earlier, and a burst of warmup
    matmuls on a zeroed tile un-throttles the PE clock (HAM) before the
    first real matmul arrives.
  - Host combines: y[t] = sum over the token's 2 experts of
    sel_weight * expert_out.
"""

import os
import sys
import functools

for _p in ("/opt/trn_rl_repo", "/root/.axon_site/_ro/trn_rl_repo"):
    if os.path.isdir(_p) and _p not in sys.path:
        sys.path.append(_p)

import numpy as np

T, H, F, E, KTOP = 2048, 1024, 1024, 16, 2
NCORES = 8
EPC = E // NCORES  # experts per core
P = 128
KO = H // P  # contraction chunks per 1024-dim
FB = F // P  # 128-blocks of F
HB = H // P  # 128-blocks of H

# "f16"  = fp16 operands (half the weight DMA bytes, full-rate PE)
# "f32r" = float32r single-pass PE mode
# "f32"  = exact fp32 PE mode (4x slower)
MM_MODE = os.environ.get("MOE_MM_MODE", "f16")
CAP_ALIGN = int(os.environ.get("MOE_CAP_ALIGN", "16"))
WDB = int(os.environ.get("MOE_WDB", "4"))      # wd-tile ring depth
W4B = int(os.environ.get("MOE_W4B", "3"))      # 2MB-wgu-tile ring depth
NWARM = int(os.environ.get("MOE_WARM", "38"))  # HAM warmup matmuls
YF32 = os.environ.get("MOE_Y_F32", "0") == "1"
YB = 2  # y h-blocks per output DMA

LAST_RESULTS = None  # BassKernelResults of the most recent device run


def _chunks(cap):
    """Split cap into moving-dim chunks <= 512 (PSUM bank / fp32 AP limit)."""
    out, rem, n = [], cap, -(-cap // 512)
    for i in range(n):
        c = min(512, rem, -(-rem // ((n - i) * 64)) * 64)
        out.append(c)
        rem -= c
    assert sum(out) == cap and all(0 < c <= 512 for c in out), (cap, out)
    return out


@functools.lru_cache(maxsize=4)
def _build_program(caps):
    import concourse.mybir as mybir
    import concourse.tile as tile
    from concourse import bacc

    f32 = mybir.dt.float32
    mm_dt = {"f16": mybir.dt.float16,
             "f32r": mybir.dt.float32r,
             "f32": f32}[MM_MODE]
    y_dt = f32 if YF32 else mybir.dt.float16
    silu = mybir.ActivationFunctionType.Silu

    nc = bacc.Bacc("TRN2", target_bir_lowering=False, debug=False,
                   num_devices=NCORES)

    xt_d, wgu_d, wd_d, yt_d = [], [], [], []
    for s in range(EPC):
        cap = caps[s]
        xt_d.append(nc.dram_tensor(f"xt{s}", [P, KO, cap], mm_dt,
                                   kind="ExternalInput").ap())
        # gate and up weights interleaved per f-block: one DMA pulls both
        wgu_d.append(nc.dram_tensor(f"wgu{s}", [P, FB, 2, H], mm_dt,
                                    kind="ExternalInput").ap())
        wd_d.append(nc.dram_tensor(f"wd{s}", [P, HB, F], mm_dt,
                                   kind="ExternalInput").ap())
        yt_d.append(nc.dram_tensor(f"yt{s}", [HB, P, cap], y_dt,
                                   kind="ExternalOutput").ap())

    def mm(ps, lhsT, rhs, start, stop):
        nc.tensor.matmul(ps, lhsT=lhsT, rhs=rhs, start=start, stop=stop)

    capmax = max(caps)
    with tile.TileContext(nc) as tc:
        with (
            tc.tile_pool(name="xp", bufs=2) as xp,
            tc.tile_pool(name="wp", bufs=2) as wp,
            tc.tile_pool(name="sp", bufs=6) as sp,
            tc.tile_pool(name="hp", bufs=2) as hp,
            tc.tile_pool(name="op", bufs=4) as op,
            tc.tile_pool(name="pp", bufs=8, space="PSUM") as pp,
        ):
            # HAM warmup: zeroed fp16 tile, back-to-back small matmuls keep
            # the PE busy while the first weights stream in, so the clock
            # gate opens before real work starts.
            if NWARM:
                wz = wp.tile([P, P], mm_dt, tag="wz", bufs=1, name="wz")
                nc.gpsimd.memset(wz, 0)
                psz = pp.tile([P, P], f32, tag="ps", name="psz")
                for _ in range(NWARM):
                    mm(psz, wz, wz, True, True)

            for s in range(EPC):
                cap = caps[s]
                cols = _chunks(cap)
                col_off = [0]
                for c in cols:
                    col_off.append(col_off[-1] + c)
                xt = xp.tile([P, KO, capmax], mm_dt, tag="xt", name=f"xt{s}")
                xt = xt[:, :, :cap]
                if s == 0:
                    # split so the first psg matmuls can start on k0-1 while
                    # the rest of x streams in
                    nc.sync.dma_start(xt[:, :2], xt_d[s][:, :2])
                    nc.sync.dma_start(xt[:, 2:], xt_d[s][:, 2:])
                else:
                    nc.sync.dma_start(xt, xt_d[s])
                h_sb = hp.tile([P, FB, capmax], mm_dt, tag="h", name=f"h{s}")
                h_sb = h_sb[:, :, :cap]
                # weight loads go on the sync HWDGE ring in just-in-time
                # program order; the scalar ring only carries y writes,
                # so head-critical bytes are never competed with (the 16
                # SDMA engines round-robin between the rings).
                fgroups = ([[0], [1], [2], [3], [4, 5], [6, 7]] if s == 0
                           else [[0, 1, 2, 3], [4, 5, 6, 7]])
                w_bufs = {2: 4, 4: W4B}
                for gi, fg in enumerate(fgroups):
                    nf = len(fg)
                    f0 = fg[0]
                    if nf == 1:
                        # early f-blocks load wg and wu as separate DMAs:
                        # smaller critical sets and overlapped completion
                        # receipts while the DMA path is still cold
                        wg0t = wp.tile([P, KO, P], mm_dt, tag="w1", bufs=8,
                                       name="wg0t")
                        nc.sync.dma_start(
                            wg0t, wgu_d[s][:, f0, 0].rearrange(
                                "p (ko m) -> p ko m", m=P))
                        wu0t = wp.tile([P, KO, P], mm_dt, tag="w1", bufs=8,
                                       name="wu0t")
                        nc.sync.dma_start(
                            wu0t, wgu_d[s][:, f0, 1].rearrange(
                                "p (ko m) -> p ko m", m=P))
                        wguf = None
                    else:
                        wguf = wp.tile([P, nf, 2, KO, P], mm_dt, tag=f"w{nf}",
                                       bufs=w_bufs[nf], name="wguf")
                        nc.sync.dma_start(
                            wguf, wgu_d[s][:, f0:f0 + nf].rearrange(
                                "p f w (ko m) -> p f w ko m", m=P))
                    for fj, f in enumerate(fg):
                        if wguf is None:
                            wgf = wg0t
                            wuf = wu0t
                        else:
                            wgf = wguf[:, fj, 0]
                            wuf = wguf[:, fj, 1]
                        for ci, ncol in enumerate(cols):
                            c0, c1 = col_off[ci], col_off[ci + 1]
                            psg = pp.tile([P, ncol], f32, tag="ps", name="psg")
                            psu = pp.tile([P, ncol], f32, tag="ps", name="psu")
                            for k in range(KO):
                                mm(psg, wgf[:, k], xt[:, k, c0:c1], k == 0, k == KO - 1)
                            for k in range(KO):
                                mm(psu, wuf[:, k], xt[:, k, c0:c1], k == 0, k == KO - 1)
                            sg = sp.tile([P, ncol], f32, tag="sg", name="sg")
                            nc.scalar.activation(sg, psg, silu)
                            nc.vector.tensor_mul(out=h_sb[:, f, c0:c1], in0=sg, in1=psu)
                # down projection: y[hb] = sum_f wd[f,hb]^T @ h[f]
                for hb0 in range(0, HB, 4):
                    wdf = wp.tile([P, 4, FB, P], mm_dt, tag="wd", bufs=WDB,
                                  name="wdf")
                    nc.sync.dma_start(
                        wdf, wd_d[s][:, hb0:hb0 + 4].rearrange(
                            "p h (fb m) -> p h fb m", m=P))
                    for hj in range(4):
                        hb = hb0 + hj
                        if len(cols) == 1:
                            ncol = cols[0]
                            psy = pp.tile([P, ncol], f32, tag="ps", name="psy")
                            for f in range(FB):
                                mm(psy, wdf[:, hj, f], h_sb[:, f], f == 0, f == FB - 1)
                            # last expert's last pair writes per-hb so the
                            # final (end-blocking) DMA is half the size
                            solo = s == EPC - 1 and hb >= HB - YB
                            if hb % YB == 0:
                                ysb = op.tile([P, YB, capmax], y_dt, tag="y",
                                              name="ysb")
                            nc.vector.tensor_copy(out=ysb[:, hb % YB, :cap], in_=psy)
                            if solo:
                                nc.scalar.dma_start(
                                    yt_d[s][hb:hb + 1].rearrange("h p c -> p h c"),
                                    ysb[:, hb % YB:hb % YB + 1, :cap])
                            elif hb % YB == YB - 1:
                                nc.scalar.dma_start(
                                    yt_d[s][hb - YB + 1:hb + 1].rearrange(
                                        "h p c -> p h c"),
                                    ysb[:, :, :cap])
                        else:
                            for ci, ncol in enumerate(cols):
                                c0, c1 = col_off[ci], col_off[ci + 1]
                                psy = pp.tile([P, ncol], f32, tag="ps", name="psy")
                                for f in range(FB):
                                    mm(psy, wdf[:, hj, f], h_sb[:, f, c0:c1],
                                       f == 0, f == FB - 1)
                                ysb = op.tile([P, ncol], y_dt, tag="y", name="ysb")
                                nc.vector.tensor_copy(out=ysb, in_=psy)
                                nc.scalar.dma_start(yt_d[s][hb, :, c0:c1], ysb)

    nc.compile()
    return nc


def _route_np(x, gate_w, bias):
    """Numpy fallback router (same math, host BLAS numerics)."""
    gates = x.astype(np.float32) @ gate_w.T
    orig = 1.0 / (1.0 + np.exp(-gates))
    corrected = orig + bias
    inds = np.argsort(-corrected, axis=-1, kind="stable")[:, :KTOP].astype(np.int32)
    sel = np.take_along_axis(orig, inds, axis=-1)
    sel = sel / (sel.sum(axis=-1, keepdims=True) + 1e-20)
    return inds, sel.astype(np.float32)


def _route(x, gate_w, bias):
    """Top-2 routing with exactly the reference's jax ops on CPU."""
    try:
        import jax
        import jax.numpy as jnp
        cpu = jax.devices("cpu")[0]
    except Exception:
        return _route_np(x, gate_w, bias)
    with jax.default_device(cpu):
        xd = jax.device_put(x, cpu)
        gd = jax.device_put(gate_w, cpu)
        bd = jax.device_put(bias, cpu)
        gates = jnp.einsum("th,eh->te", xd.astype(jnp.float32), gd)
        orig = jax.nn.sigmoid(gates)
        corrected = orig + bd
        _, inds = jax.lax.top_k(corrected, KTOP)
        sel = jnp.take_along_axis(orig, inds, axis=-1)
        sel = sel / (jnp.sum(sel, axis=-1, keepdims=True) + 1e-20)
        sel = sel.astype(x.dtype)
    return np.asarray(inds), np.asarray(sel)


_PACK_CACHE = {}


NP_MM_DT = np.float16 if MM_MODE == "f16" else np.float32
NP_Y_DT = np.float32 if YF32 else np.float16


def _pack(w):
    """[1024, 1024] -> [128, 8, 1024]: out[p, b, k*128+m] = w[k*128+p, b*128+m].

    Partition-major so a [p, f0:f1] DMA slice is one contiguous multi-KB
    run per partition (big DMA descriptors)."""
    return np.ascontiguousarray(
        w.reshape(8, P, 8, P).transpose(1, 2, 0, 3).reshape(P, 8, 8 * P)
        .astype(NP_MM_DT))


def _ensure_ntff_hook():
    """Best-effort: register the axon NTFF profile hook if this image's
    antenv lacks it, so BASS_TRACE=1 yields exec_time_ns.  No-op when the
    module already exists (e.g. the harness registered its own)."""
    try:
        import importlib
        try:
            importlib.import_module("antenv.axon_hooks")
            return
        except ImportError:
            pass
        import types
        import antenv

        mod = types.ModuleType("antenv.axon_hooks")
        _hook = [None]
        mod.set_axon_ntff_profile_hook = lambda h: _hook.__setitem__(0, h)
        mod.get_axon_ntff_profile_hook = lambda: _hook[0]
        sys.modules["antenv.axon_hooks"] = mod
        antenv.axon_hooks = mod
        if "/root/.axon_site" not in sys.path:
            sys.path.insert(0, "/root/.axon_site")
        from trn_agent_boot.trn_boot import _ntff_profile_via_ctypes

        mod.set_axon_ntff_profile_hook(
            _ntff_profile_via_ctypes("/opt/axon/libaxon_pjrt.so"))
    except Exception:
        pass


def kernel(x, gate_w, w_gate, w_up, w_down, e_score_correction_bias):
    global LAST_RESULTS
    _ensure_ntff_hook()
    from concourse import bass_utils

    x = np.asarray(x, dtype=np.float32)
    inds, sel = _route(x, np.asarray(gate_w, np.float32),
                       np.asarray(e_score_correction_bias, np.float32))

    # dispatch: token lists per expert
    tok_idx, tok_w = [], []
    for e in range(E):
        rows, slots = np.nonzero(inds == e)
        tok_idx.append(rows)
        tok_w.append(sel[rows, slots])
    counts = np.array([len(t) for t in tok_idx])

    # Pair heavy experts with light ones: slot 0 of each core gets one of
    # the 8 largest experts, slot 1 one of the 8 smallest, so slot 1's
    # capacity (max over its experts) can be smaller than slot 0's.
    order = np.argsort(-counts, kind="stable")
    assign = [(int(order[c]), int(order[E - 1 - c])) for c in range(NCORES)]

    def _cap(n):
        if MM_MODE == "f16":
            return max(64, -(-max(n, 1) // CAP_ALIGN) * CAP_ALIGN)
        return max(256, -(-max(n, 1) // 64) * 64)

    caps = tuple(_cap(int(counts[[assign[c][s] for c in range(NCORES)]].max()))
                 for s in range(EPC))

    nc = _build_program(caps)

    # weight packing (cached on the weight buffers' identity)
    wkey = (id(w_gate), id(w_up), id(w_down),
            w_gate.shape if hasattr(w_gate, "shape") else None)
    packed = _PACK_CACHE.get(wkey)
    if packed is None:
        wg = np.asarray(w_gate, np.float32)
        wu = np.asarray(w_up, np.float32)
        wd = np.asarray(w_down, np.float32)
        # [P, FB, 2, H]: gate and up interleaved per f-block
        wgu_p = [np.ascontiguousarray(
                     np.stack([_pack(wg[e]), _pack(wu[e])], axis=2))
                 for e in range(E)]
        packed = (wgu_p, [_pack(wd[e]) for e in range(E)])
        _PACK_CACHE.clear()
        _PACK_CACHE[wkey] = packed
    wgu_p, wd_p = packed

    in_maps = []
    for c in range(NCORES):
        m = {}
        for s in range(EPC):
            e = assign[c][s]
            xt = np.zeros((P, KO, caps[s]), NP_MM_DT)
            cnt = len(tok_idx[e])
            if cnt:
                g = x[tok_idx[e]].astype(NP_MM_DT)  # [cnt, H]
                xt[:, :, :cnt] = g.reshape(cnt, KO, P).transpose(2, 1, 0)
            m[f"xt{s}"] = xt
            m[f"wgu{s}"] = wgu_p[e]
            m[f"wd{s}"] = wd_p[e]
        in_maps.append(m)

    res = None
    last_err = None
    for attempt in range(3):
        try:
            res = bass_utils.run_bass_kernel_spmd(
                nc, in_maps, core_ids=list(range(NCORES)))
            break
        except Exception as err:  # transient NRT/device errors happen
            last_err = err
            import time as _time
            _time.sleep(3.0 * (attempt + 1))
    if res is None:
        raise last_err
    LAST_RESULTS = res

    y = np.zeros((x.shape[0], H), np.float32)
    for c in range(NCORES):
        for s in range(EPC):
            e = assign[c][s]
            cnt = len(tok_idx[e])
            if not cnt:
                continue
            yt = res.results[c][f"yt{s}"].reshape(H, caps[s]).astype(np.float32)
            y[tok_idx[e]] += tok_w[e][:, None] * yt[:, :cnt].T
    return y
